# revision 15
# baseline (speedup 1.0000x reference)
"""Distributed multi-head attention (RoPE, non-causal) on 8 TRN2 NeuronCores.

Sharding: tensor-parallel over heads. Core c owns heads {2c, 2c+1}:
  - wq/wk/wv rows c*256:(c+1)*256 (output dim), x replicated (pre-transposed),
  - attention computed locally per (batch, head),
  - per-chunk AllGather of the attention outputs (transposed layout, bf16),
  - each core then computes output columns c*256:(c+1)*256 with its wo rows.
Host side only shards/casts inputs and concatenates the 8 output column
slices -- all FLOPs run on device.

v2 schedule: one globally interleaved emission stream per batch window.
Window b runs attention(b) in 4-matmul "steps" (2 score pairs feeding one
[128,1024] exp + one 16-matmul attn@v group of the previous unit + the
one-step-deferred transpose), with proj(b+1) and wo(b-1) chains paced in
between as scalar-free filler.  This keeps TensorE continuously fed (its
clock drops after any stall) while ScalarE (exp-bound during attention)
runs at ~40% duty instead of gating the PE.

Layout/precision tricks (as v1):
  - All matmuls bf16, PSUM accumulates f32; rel-err ~5e-3.
  - RoPE pairs separated into halves by permuting wq/wk rows on the host.
  - Scores computed transposed [k, q]; softmax denominator from a ones
    column appended to v (matmul N=129); no max-subtraction.
  - attn output normalized per-partition then PE-transposed to [hd, q].
  - wo computes out.T; host transposes back.
New in v2:
  - exp runs on [128,1024] PSUM pairs (two scores matmuls per tile).
  - transposes deferred one group so the PE never waits on Vector.
  - bounce/gather/out DMAs on the sync queue (scalar does only exp).
  - startup DMAs chunked so the first matmul issues within ~5us.
  - AllGather granularity per batch: halves (b0,b1), quarters (b2,b3)
    to shrink the CC-bound tail; wo prefetches gathers on the sync queue.
"""

import numpy as np
import ml_dtypes

B, S, D, H = 4, 2048, 2048, 16
HD = 128            # head dim
NCORES = 8
HPC = H // NCORES   # heads per core = 2
OSL = HPC * HD      # per-core o-slice = 256
ROWS = B * S        # 8192 flattened rows
DCH = D // 128      # 16 contraction chunks
SCH = 512           # seq chunk for projections
KB = S // 128       # 16 k-blocks per batch
QC = 512            # q chunk in attention
NQC = S // QC       # 4

# AllGather chunk widths per batch (cols of the [OSL, S] attn output).
# CC ops have ~20us fixed cost -> few big AGs; last batch ends with a small
# chunk so the tail-critical final AG is short.
CHUNKS = [[1024, 1024], [1024, 1024], [512, 512, 512, 512],
          [512, 512, 512, 512]]


def _chunk_of(b, qc):
    """quarter qc (512 cols) -> (chunk idx, col offset, is-last-quarter)."""
    col = qc * QC
    acc = 0
    for ch, w in enumerate(CHUNKS[b]):
        if col < acc + w:
            return ch, col - acc, (col + QC == acc + w)
        acc += w
    raise ValueError((b, qc))

BF16 = ml_dtypes.bfloat16
_NC_CACHE = None


def _build():
    import concourse.bass as bass  # noqa: F401
    import concourse.mybir as mybir
    import concourse.tile as tile
    from concourse import bacc
    from concourse.masks import make_identity

    fp32 = mybir.dt.float32
    bf16 = mybir.dt.bfloat16

    nc = bacc.Bacc(
        "TRN2",
        target_bir_lowering=False,
        debug=False,
        num_devices=NCORES,
    )

    xT = nc.declare_dram_parameter("xT", [D, ROWS], bf16, isOutput=False)
    wqT = nc.declare_dram_parameter("wqT", [D, OSL], bf16, isOutput=False)
    wkT = nc.declare_dram_parameter("wkT", [D, OSL], bf16, isOutput=False)
    wvT = nc.declare_dram_parameter("wvT", [D, OSL], bf16, isOutput=False)
    woT = nc.declare_dram_parameter("woT", [D, OSL], bf16, isOutput=False)
    cosd = nc.declare_dram_parameter("cosd", [128, S], fp32, isOutput=False)
    sind = nc.declare_dram_parameter("sind", [128, S], fp32, isOutput=False)
    outp = nc.declare_dram_parameter("out", [OSL, ROWS], fp32, isOutput=True)

    inv_sqrt_hd = 1.0 / float(np.sqrt(HD))

    with tile.TileContext(nc) as tc:
        with (
            tc.tile_pool(name="glob", bufs=1) as glob,
            tc.tile_pool(name="dram", bufs=1, space="DRAM") as dram,
            tc.tile_pool(name="qkv", bufs=2) as qkv,
            tc.tile_pool(name="xtp", bufs=4) as xtp,
            tc.tile_pool(name="attp", bufs=4) as attp,
            tc.tile_pool(name="gtp", bufs=4) as gtp,
            tc.tile_pool(name="tmpp", bufs=1) as tmpp,
            tc.tile_pool(name="smalls", bufs=2) as smalls,
            tc.tile_pool(name="atp", bufs=4) as atp,
            tc.tile_pool(name="otp", bufs=2) as otp,
            tc.tile_pool(name="psA", bufs=2, space="PSUM") as psA,
            tc.tile_pool(name="psB", bufs=2, space="PSUM") as psB,
            tc.tile_pool(name="psCD", bufs=2, space="PSUM") as psCD,
        ):
            ident = glob.tile([128, 128], bf16, name="ident")

            # weights as two half-tiles each so the first proj chain can
            # start as soon as the first 512KB lands
            wq_sb = [glob.tile([128, DCH // 2, OSL], bf16, name=f"wq{i}")
                     for i in range(2)]
            wk_sb = [glob.tile([128, DCH // 2, OSL], bf16, name=f"wk{i}")
                     for i in range(2)]
            wv_sb = [glob.tile([128, DCH // 2, OSL], bf16, name=f"wv{i}")
                     for i in range(2)]
            wo_sb = [glob.tile([128, DCH // 2, OSL], bf16, name=f"wo{i}")
                     for i in range(2)]
            cosb = glob.tile([128, S], fp32, name="cosb")
            sinb = glob.tile([128, S], fp32, name="sinb")

            xstate = {}

            def fetch_x(b, sc):
                col0 = b * S + sc * SCH
                xh = []
                for half in range(2):
                    xth = xtp.tile([128, DCH // 2, SCH], bf16,
                                   name=f"xt{half}", tag="xt")
                    nc.gpsimd.dma_start(
                        xth[:],
                        xT[half * 1024:(half + 1) * 1024, col0:col0 + SCH]
                        .rearrange("(c p) n -> p c n", p=128))
                    xh.append(xth)
                xstate[(b, sc)] = xh

            # startup order: what the first projection chain needs, first
            def wdma(dst, src, half):
                nc.gpsimd.dma_start(
                    dst[half][:],
                    src[half * 1024:(half + 1) * 1024, :]
                    .rearrange("(c p) n -> p c n", p=128))

            wdma(wk_sb, wkT, 0)
            fetch_x(0, 0)
            make_identity(nc, ident[:])
            wdma(wk_sb, wkT, 1)
            fetch_x(0, 1)
            nc.gpsimd.dma_start(cosb[:, 0:SCH], cosd[:, 0:SCH])
            nc.gpsimd.dma_start(sinb[:, 0:SCH], sind[:, 0:SCH])
            wdma(wq_sb, wqT, 0)
            wdma(wq_sb, wqT, 1)
            nc.gpsimd.dma_start(cosb[:, SCH:2 * SCH], cosd[:, SCH:2 * SCH])
            nc.gpsimd.dma_start(sinb[:, SCH:2 * SCH], sind[:, SCH:2 * SCH])
            wdma(wv_sb, wvT, 0)
            wdma(wv_sb, wvT, 1)
            for sc in (2, 3):
                cs = slice(sc * SCH, (sc + 1) * SCH)
                nc.gpsimd.dma_start(cosb[:, cs], cosd[:, cs])
                nc.gpsimd.dma_start(sinb[:, cs], sind[:, cs])

            bounce = {}
            gath = {}
            for b in range(B):
                for ch, w in enumerate(CHUNKS[b]):
                    bounce[(b, ch)] = dram.tile(
                        [OSL, w], bf16, name=f"bounce{b}_{ch}")
                    gath[(b, ch)] = dram.tile(
                        [NCORES * OSL, w], bf16, addr_space="Shared",
                        name=f"gath{b}_{ch}")

            # ---------- projection chains ----------
            qkvstate = {}

            def get_qkv(b):
                if b not in qkvstate:
                    qt = qkv.tile([128, HPC, S], bf16, name="qt", tag="qt")
                    kt = qkv.tile([128, HPC, S], bf16, name="kt", tag="kt")
                    vt = qkv.tile([128, KB, HPC, HD + 1], bf16, name="vt",
                                  tag="vt")
                    qkvstate[b] = (qt, kt, vt)
                return qkvstate[b]

            def qk_chain(b, sc, w_sb, dst_idx, h):
                qt, kt, vt = get_qkv(b)
                dstT = (qt, kt)[dst_idx]
                xh = xstate[(b, sc)]
                cosr = cosb[:, sc * SCH:(sc + 1) * SCH]
                sinr = sinb[:, sc * SCH:(sc + 1) * SCH]
                ps = psA.tile([128, SCH], fp32, name="ps_proj", tag="psA")
                for c in range(DCH):
                    nc.tensor.matmul(
                        ps[:],
                        w_sb[c // 8][:, c % 8, h * HD:(h + 1) * HD],
                        xh[c // 8][:, c % 8, :],
                        start=(c == 0), stop=(c == DCH - 1))
                m1 = tmpp.tile([128, SCH], fp32, name="m1", tag="m1")
                m2 = tmpp.tile([128, SCH], fp32, name="m2", tag="m2")
                nc.vector.tensor_mul(m1[:], ps[:], cosr)
                nc.vector.tensor_mul(
                    m2[0:64, :], ps[64:128, :], sinr[0:64, :])
                nc.vector.tensor_mul(
                    m2[64:128, :], ps[0:64, :], sinr[64:128, :])
                sl = slice(sc * SCH, (sc + 1) * SCH)
                nc.vector.tensor_sub(
                    dstT[0:64, h, sl], m1[0:64, :], m2[0:64, :])
                nc.vector.tensor_add(
                    dstT[64:128, h, sl], m2[64:128, :], m1[64:128, :])

            vt_init = set()

            def v_chain(b, sc, ssb):
                qt, kt, vt = get_qkv(b)
                if b not in vt_init:
                    vt_init.add(b)
                    nc.vector.memset(vt[:, :, :, HD:HD + 1], 1.0)
                xh = xstate[(b, sc)]
                kb = sc * (SCH // 128) + ssb
                psv = psA.tile([128, OSL], fp32, name="psv", tag="psA")
                for c in range(DCH):
                    nc.tensor.matmul(
                        psv[:],
                        xh[c // 8][:, c % 8, ssb * 128:(ssb + 1) * 128],
                        wv_sb[c // 8][:, c % 8, :],
                        start=(c == 0), stop=(c == DCH - 1))
                nc.vector.tensor_copy(
                    vt[:, kb, :, 0:HD],
                    psv[:].rearrange("p (h d) -> p h d", h=HPC))

            def proj_fillers(b):
                """Closure list emitting proj(b): fetches + chains."""
                items = []
                for sc in range(S // SCH):
                    if (b, sc) not in xstate:
                        items.append(lambda b=b, sc=sc: fetch_x(b, sc))
                    for h in range(HPC):
                        items.append(
                            lambda b=b, sc=sc, h=h: qk_chain(b, sc, wk_sb, 1, h))
                    for h in range(HPC):
                        items.append(
                            lambda b=b, sc=sc, h=h: qk_chain(b, sc, wq_sb, 0, h))
                    if sc + 1 < S // SCH:
                        items.append(lambda b=b, sc=sc: fetch_x(b, sc + 1))
                    elif b + 1 < B:
                        items.append(lambda b=b: fetch_x(b + 1, 0))
                    for ssb in range(SCH // 128):
                        items.append(
                            lambda b=b, sc=sc, ssb=ssb: v_chain(b, sc, ssb))
                return items

            # ---------- attention ----------
            expstate = {}
            atstate = {}

            def score_pair(b, u, j):
                """Two scores matmuls into one 2-bank PSUM tile + one exp."""
                qt, kt, vt = get_qkv(b)
                h, qc = u % HPC, u // HPC
                eh, jj = j // 4, j % 4
                if (u % 2, eh) not in expstate or expstate[(u % 2, eh)][0] != (b, u):
                    t = attp.tile([128, KB // 2, QC], bf16, name="expT",
                                  tag="expT")
                    expstate[(u % 2, eh)] = ((b, u), t)
                expT = expstate[(u % 2, eh)][1]
                pss = psB.tile([128, 2 * QC], fp32, name="pss", tag="psB")
                for i in range(2):
                    kb = eh * (KB // 2) + 2 * jj + i
                    nc.tensor.matmul(
                        pss[:, i * QC:(i + 1) * QC],
                        kt[:, h, kb * 128:(kb + 1) * 128],
                        qt[:, h, qc * QC:(qc + 1) * QC],
                        start=True, stop=True)
                nc.scalar.activation(
                    expT[:, 2 * jj:2 * jj + 2, :],
                    pss[:].rearrange("p (a n) -> p a n", a=2),
                    mybir.ActivationFunctionType.Exp,
                    scale=inv_sqrt_hd)

            def attnv_mms(b, u, s):
                """16 accumulating matmuls for q-subblock s of unit u."""
                qt, kt, vt = get_qkv(b)
                h = u % HPC
                e0 = expstate[(u % 2, 0)][1]
                e1 = expstate[(u % 2, 1)][1]
                if (b, u) not in atstate:
                    atstate[(b, u)] = atp.tile([128, QC], bf16, name="a_t",
                                               tag="a_t")
                a_t = atstate[(b, u)]
                pso = psCD.tile([128, HD + 1], fp32, name="pso", tag="psCD")
                for kb in range(KB):
                    eT = e0 if kb < KB // 2 else e1
                    nc.tensor.matmul(
                        pso[:],
                        eT[:, kb % (KB // 2), s * 128:(s + 1) * 128],
                        vt[:, kb, h, :],
                        start=(kb == 0), stop=(kb == KB - 1))
                return pso, a_t

            def finish(b, u, s, pso, a_t):
                """normalize + transpose + copy (+ bounce DMA / AG)."""
                h, qc = u % HPC, u // HPC
                rc = smalls.tile([128, 1], fp32, name="rc", tag="rc")
                nc.vector.reciprocal(rc[:], pso[:, HD:HD + 1])
                a_sb = smalls.tile([128, HD], bf16, name="a_sb", tag="a_sb")
                nc.vector.tensor_scalar_mul(a_sb[:], pso[:, 0:HD], rc[:])
                pst = psCD.tile([128, 128], bf16, name="pst", tag="psCD")
                nc.tensor.transpose(pst[:], a_sb[:], ident[:])
                nc.vector.tensor_copy(a_t[:, s * 128:(s + 1) * 128], pst[:])
                if s == 3:
                    ch, col0, last = _chunk_of(b, qc)
                    nc.sync.dma_start(
                        bounce[(b, ch)][h * HD:(h + 1) * HD, col0:col0 + QC],
                        a_t[:])
                    if h == HPC - 1 and last:
                        nc.gpsimd.collective_compute(
                            "AllGather",
                            mybir.AluOpType.bypass,
                            ins=[bounce[(b, ch)].opt()],
                            outs=[gath[(b, ch)].opt()],
                            replica_groups=[list(range(NCORES))],
                        )

            # ---------- wo stage ----------
            gtstate = {}

            def wo_prefetch(b, qq):
                """Load gathered [2048, 512] cols qq*512.. into 2 SBUF tiles."""
                ch, off, _ = _chunk_of(b, qq)
                gh = []
                for dh in range(2):
                    g = gtp.tile([128, DCH // 2, QC], bf16, name=f"gt{dh}",
                                 tag="gt")
                    nc.sync.dma_start(
                        g[:],
                        gath[(b, ch)][dh * 1024:(dh + 1) * 1024,
                                      off:off + QC]
                        .rearrange("(c p) n -> p c n", p=128))
                    gh.append(g)
                gtstate[(b, qq)] = gh

            def wo_chain(b, qq, oc):
                gh = gtstate[(b, qq)]
                psw = psA.tile([128, QC], fp32, name="psw", tag="psA")
                for c in range(DCH):
                    nc.tensor.matmul(
                        psw[:],
                        wo_sb[c // 8][:, c % 8, oc * 128:(oc + 1) * 128],
                        gh[c // 8][:, c % 8, :],
                        start=(c == 0), stop=(c == DCH - 1))
                out_t = otp.tile([128, QC], fp32, name="out_t", tag="out_t")
                nc.vector.tensor_copy(out_t[:], psw[:])
                col0 = b * S + qq * QC
                nc.scalar.dma_start(
                    outp[oc * 128:(oc + 1) * 128, col0:col0 + QC], out_t[:])

            def wo_fillers(b):
                items = [lambda b=b: wo_prefetch(b, 0),
                         lambda b=b: wo_prefetch(b, 1)]
                for qq in range(4):
                    for oc in range(OSL // 128):
                        items.append(
                            lambda b=b, qq=qq, oc=oc: wo_chain(b, qq, oc))
                    if qq + 2 < 4:
                        items.append(
                            lambda b=b, qq=qq: wo_prefetch(b, qq + 2))
                return items

            # ---------- the interleaved window ----------
            pending = [None]
            deferred_wo = []

            def flush_pending():
                if pending[0] is not None:
                    fin = pending[0]
                    pending[0] = None
                    fin()

            def window(b):
                fillers = []
                if b + 1 < B:
                    fillers.extend((0, f) for f in proj_fillers(b + 1))
                if b >= 1:
                    wof = wo_fillers(b - 1)
                    if b == B - 1:
                        deferred_wo.extend(wof[-2:])
                        wof = wof[:-2]
                    fillers.extend((0, f) for f in wof)
                nf = len(fillers)
                nsteps = 9 * 4
                # last window: hold fillers until step 8 so the first
                # attention units (and their bounce->AllGather chain, the
                # tail critical path) complete as early as possible
                s0 = 8 if b == B - 1 else 0
                emitted = [0]

                def pace(step):
                    es = max(0, step - s0 + 1)
                    target = (nf * es + (nsteps - s0) - 1) // (nsteps - s0)
                    while (emitted[0] < min(target, nf)
                           and fillers[emitted[0]][0] <= step):
                        fillers[emitted[0]][1]()
                        emitted[0] += 1

                step = 0
                for u in range(9):
                    for s in range(4):
                        if u < 8:
                            score_pair(b, u, 2 * s)
                            score_pair(b, u, 2 * s + 1)
                        if u >= 1:
                            pso, a_t = attnv_mms(b, u - 1, s)
                            flush_pending()
                            pending[0] = (
                                lambda b=b, u=u - 1, s=s, pso=pso, a_t=a_t:
                                finish(b, u, s, pso, a_t))
                        elif u == 0 and s == 0:
                            # previous window's last finish, after 2 pairs
                            flush_pending()
                        pace(step)
                        step += 1

            # emission: proj(0) standalone, then the 4 windows, then wo(3)
            p0 = proj_fillers(0)
            for i, it in enumerate(p0):
                it()
                if i == 8:
                    wdma(wo_sb, woT, 0)
                elif i == 17:
                    wdma(wo_sb, woT, 1)
            for b in range(B):
                window(b)
            flush_pending()            # finish(7, 3) of batch 3 + its AG
            for f in deferred_wo:      # held-back wo(2) chains fill AG wait
                f()
            wo_prefetch(3, 0)
            wo_prefetch(3, 1)
            for oc in range(OSL // 128):
                wo_chain(3, 0, oc)
            wo_prefetch(3, 2)
            for oc in range(OSL // 128):
                wo_chain(3, 1, oc)
            wo_prefetch(3, 3)
            for qq in (2, 3):
                for oc in range(OSL // 128):
                    wo_chain(3, qq, oc)

    nc.compile()
    return nc


def _shard_inputs(x, freqs_cos, freqs_sin, wq, wk, wv, wo):
    xf = np.asarray(x, dtype=np.float32).reshape(ROWS, D)
    xT = np.ascontiguousarray(xf.T).astype(BF16)
    fcT = np.asarray(freqs_cos, dtype=np.float32).T  # [64, S]
    fsT = np.asarray(freqs_sin, dtype=np.float32).T
    cosd = np.ascontiguousarray(np.concatenate([fcT, fcT], 0))  # [128, S]
    sind = np.ascontiguousarray(np.concatenate([fsT, fsT], 0))
    # even indices (real half) then odd (imag half), per head
    perm = np.concatenate([np.arange(0, HD, 2), np.arange(1, HD, 2)])
    in_maps = []
    for c in range(NCORES):
        rows = slice(c * OSL, (c + 1) * OSL)
        wq_c = np.asarray(wq)[rows].reshape(HPC, HD, D)[:, perm, :].reshape(OSL, D)
        wk_c = np.asarray(wk)[rows].reshape(HPC, HD, D)[:, perm, :].reshape(OSL, D)
        in_maps.append({
            "xT": xT,
            "wqT": np.ascontiguousarray(wq_c.T).astype(BF16),
            "wkT": np.ascontiguousarray(wk_c.T).astype(BF16),
            "wvT": np.ascontiguousarray(np.asarray(wv)[rows].T).astype(BF16),
            "woT": np.ascontiguousarray(np.asarray(wo)[rows].T).astype(BF16),
            "cosd": cosd,
            "sind": sind,
        })
    return in_maps


def run(inputs, trace=False, trace_cores=None):
    """Build (cached), run on 8 cores; returns (full_output, BassKernelResults)."""
    global _NC_CACHE
    from concourse.bass_utils import run_bass_kernel_spmd
    if _NC_CACHE is None:
        _NC_CACHE = _build()
    in_maps = _shard_inputs(**inputs)
    res = run_bass_kernel_spmd(
        _NC_CACHE, in_maps, core_ids=list(range(NCORES)), trace=trace,
        trace_cores=trace_cores)
    parts = [np.ascontiguousarray(
        np.asarray(res.results[c]["out"], dtype=np.float32).T)
        for c in range(NCORES)]
    full = np.concatenate(parts, axis=1).reshape(B, S, D)
    return full, res


def kernel(x, freqs_cos, freqs_sin, wq, wk, wv, wo):
    full, _ = run(dict(x=x, freqs_cos=freqs_cos, freqs_sin=freqs_sin,
                       wq=wq, wk=wk, wv=wv, wo=wo))
    return full


# revision 16
# speedup vs baseline: 1.0113x; 1.0113x over previous
"""Distributed multi-head attention (RoPE, non-causal) on 8 TRN2 NeuronCores.

Sharding: tensor-parallel over heads. Core c owns heads {2c, 2c+1}:
  - wq/wk/wv rows c*256:(c+1)*256 (output dim), x replicated (pre-transposed),
  - attention computed locally per (batch, head),
  - per-chunk AllGather of the attention outputs (transposed layout, bf16),
  - each core then computes output columns c*256:(c+1)*256 with its wo rows.
Host side only shards/casts inputs and concatenates the 8 output column
slices -- all FLOPs run on device.

v2 schedule: one globally interleaved emission stream per batch window.
Window b runs attention(b) in 4-matmul "steps" (2 score pairs feeding one
[128,1024] exp + one 16-matmul attn@v group of the previous unit + the
one-step-deferred transpose), with proj(b+1) and wo(b-1) chains paced in
between as scalar-free filler.  This keeps TensorE continuously fed (its
clock drops after any stall) while ScalarE (exp-bound during attention)
runs at ~40% duty instead of gating the PE.

Layout/precision tricks (as v1):
  - All matmuls bf16, PSUM accumulates f32; rel-err ~5e-3.
  - RoPE pairs separated into halves by permuting wq/wk rows on the host.
  - Scores computed transposed [k, q]; softmax denominator from a ones
    column appended to v (matmul N=129); no max-subtraction.
  - attn output normalized per-partition then PE-transposed to [hd, q].
  - wo computes out.T; host transposes back.
New in v2:
  - exp runs on [128,1024] PSUM pairs (two scores matmuls per tile).
  - transposes deferred one group so the PE never waits on Vector.
  - bounce/gather/out DMAs on the sync queue (scalar does only exp).
  - startup DMAs chunked so the first matmul issues within ~5us.
  - AllGather granularity per batch: halves (b0,b1), quarters (b2,b3)
    to shrink the CC-bound tail; wo prefetches gathers on the sync queue.
"""

import numpy as np
import ml_dtypes

B, S, D, H = 4, 2048, 2048, 16
HD = 128            # head dim
NCORES = 8
HPC = H // NCORES   # heads per core = 2
OSL = HPC * HD      # per-core o-slice = 256
ROWS = B * S        # 8192 flattened rows
DCH = D // 128      # 16 contraction chunks
SCH = 512           # seq chunk for projections
KB = S // 128       # 16 k-blocks per batch
QC = 512            # q chunk in attention
NQC = S // QC       # 4

# AllGather chunk widths per batch (cols of the [OSL, S] attn output).
# CC ops have ~20us fixed cost -> few big AGs; last batch ends with a small
# chunk so the tail-critical final AG is short.
CHUNKS = [[1024, 1024], [1024, 1024], [512, 512, 512, 512],
          [512, 512, 512, 512]]


def _chunk_of(b, qc):
    """quarter qc (512 cols) -> (chunk idx, col offset, is-last-quarter)."""
    col = qc * QC
    acc = 0
    for ch, w in enumerate(CHUNKS[b]):
        if col < acc + w:
            return ch, col - acc, (col + QC == acc + w)
        acc += w
    raise ValueError((b, qc))

BF16 = ml_dtypes.bfloat16
_NC_CACHE = None


def _build():
    import concourse.bass as bass  # noqa: F401
    import concourse.mybir as mybir
    import concourse.tile as tile
    from concourse import bacc
    from concourse.masks import make_identity

    fp32 = mybir.dt.float32
    bf16 = mybir.dt.bfloat16

    nc = bacc.Bacc(
        "TRN2",
        target_bir_lowering=False,
        debug=False,
        num_devices=NCORES,
    )

    xT = nc.declare_dram_parameter("xT", [D, ROWS], bf16, isOutput=False)
    wqT = nc.declare_dram_parameter("wqT", [D, OSL], bf16, isOutput=False)
    wkT = nc.declare_dram_parameter("wkT", [D, OSL], bf16, isOutput=False)
    wvT = nc.declare_dram_parameter("wvT", [D, OSL], bf16, isOutput=False)
    woT = nc.declare_dram_parameter("woT", [D, OSL], bf16, isOutput=False)
    cosd = nc.declare_dram_parameter("cosd", [128, S], fp32, isOutput=False)
    sind = nc.declare_dram_parameter("sind", [128, S], fp32, isOutput=False)
    outp = nc.declare_dram_parameter("out", [OSL, ROWS], fp32, isOutput=True)

    inv_sqrt_hd = 1.0 / float(np.sqrt(HD))

    with tile.TileContext(nc) as tc:
        with (
            tc.tile_pool(name="glob", bufs=1) as glob,
            tc.tile_pool(name="dram", bufs=1, space="DRAM") as dram,
            tc.tile_pool(name="qkv", bufs=2) as qkv,
            tc.tile_pool(name="xtp", bufs=4) as xtp,
            tc.tile_pool(name="attp", bufs=4) as attp,
            tc.tile_pool(name="gtp", bufs=4) as gtp,
            tc.tile_pool(name="tmpp", bufs=1) as tmpp,
            tc.tile_pool(name="smalls", bufs=2) as smalls,
            tc.tile_pool(name="atp", bufs=4) as atp,
            tc.tile_pool(name="otp", bufs=2) as otp,
            tc.tile_pool(name="psA", bufs=2, space="PSUM") as psA,
            tc.tile_pool(name="psB", bufs=2, space="PSUM") as psB,
            tc.tile_pool(name="psCD", bufs=2, space="PSUM") as psCD,
        ):
            ident = glob.tile([128, 128], bf16, name="ident")

            # weights as two half-tiles each so the first proj chain can
            # start as soon as the first 512KB lands
            wq_sb = [glob.tile([128, DCH // 2, OSL], bf16, name=f"wq{i}")
                     for i in range(2)]
            wk_sb = [glob.tile([128, DCH // 2, OSL], bf16, name=f"wk{i}")
                     for i in range(2)]
            wv_sb = [glob.tile([128, DCH // 2, OSL], bf16, name=f"wv{i}")
                     for i in range(2)]
            wo_sb = [glob.tile([128, DCH // 2, OSL], bf16, name=f"wo{i}")
                     for i in range(2)]
            cosb = glob.tile([128, S], fp32, name="cosb")
            sinb = glob.tile([128, S], fp32, name="sinb")

            xstate = {}

            def fetch_x(b, sc):
                col0 = b * S + sc * SCH
                xh = []
                for half in range(2):
                    xth = xtp.tile([128, DCH // 2, SCH], bf16,
                                   name=f"xt{half}", tag="xt")
                    nc.gpsimd.dma_start(
                        xth[:],
                        xT[half * 1024:(half + 1) * 1024, col0:col0 + SCH]
                        .rearrange("(c p) n -> p c n", p=128))
                    xh.append(xth)
                xstate[(b, sc)] = xh

            # startup order: what the first projection chain needs, first
            def wdma(dst, src, half):
                nc.gpsimd.dma_start(
                    dst[half][:],
                    src[half * 1024:(half + 1) * 1024, :]
                    .rearrange("(c p) n -> p c n", p=128))

            wdma(wk_sb, wkT, 0)
            fetch_x(0, 0)
            make_identity(nc, ident[:])
            wdma(wk_sb, wkT, 1)
            nc.gpsimd.dma_start(cosb[:, 0:SCH], cosd[:, 0:SCH])
            nc.gpsimd.dma_start(sinb[:, 0:SCH], sind[:, 0:SCH])
            wdma(wq_sb, wqT, 0)
            wdma(wq_sb, wqT, 1)
            wdma(wv_sb, wvT, 0)
            wdma(wv_sb, wvT, 1)
            nc.gpsimd.dma_start(cosb[:, SCH:], cosd[:, SCH:])
            nc.gpsimd.dma_start(sinb[:, SCH:], sind[:, SCH:])

            bounce = {}
            gath = {}
            for b in range(B):
                for ch, w in enumerate(CHUNKS[b]):
                    bounce[(b, ch)] = dram.tile(
                        [OSL, w], bf16, name=f"bounce{b}_{ch}")
                    gath[(b, ch)] = dram.tile(
                        [NCORES * OSL, w], bf16, addr_space="Shared",
                        name=f"gath{b}_{ch}")

            # ---------- projection chains ----------
            qkvstate = {}

            def get_qkv(b):
                if b not in qkvstate:
                    qt = qkv.tile([128, HPC, S], bf16, name="qt", tag="qt")
                    kt = qkv.tile([128, HPC, S], bf16, name="kt", tag="kt")
                    vt = qkv.tile([128, KB, HPC, HD + 1], bf16, name="vt",
                                  tag="vt")
                    qkvstate[b] = (qt, kt, vt)
                return qkvstate[b]

            def qk_chain(b, sc, w_sb, dst_idx, h):
                qt, kt, vt = get_qkv(b)
                dstT = (qt, kt)[dst_idx]
                xh = xstate[(b, sc)]
                cosr = cosb[:, sc * SCH:(sc + 1) * SCH]
                sinr = sinb[:, sc * SCH:(sc + 1) * SCH]
                ps = psA.tile([128, SCH], fp32, name="ps_proj", tag="psA")
                for c in range(DCH):
                    nc.tensor.matmul(
                        ps[:],
                        w_sb[c // 8][:, c % 8, h * HD:(h + 1) * HD],
                        xh[c // 8][:, c % 8, :],
                        start=(c == 0), stop=(c == DCH - 1))
                m1 = tmpp.tile([128, SCH], fp32, name="m1", tag="m1")
                m2 = tmpp.tile([128, SCH], fp32, name="m2", tag="m2")
                nc.vector.tensor_mul(m1[:], ps[:], cosr)
                nc.vector.tensor_mul(
                    m2[0:64, :], ps[64:128, :], sinr[0:64, :])
                nc.vector.tensor_mul(
                    m2[64:128, :], ps[0:64, :], sinr[64:128, :])
                sl = slice(sc * SCH, (sc + 1) * SCH)
                nc.vector.tensor_sub(
                    dstT[0:64, h, sl], m1[0:64, :], m2[0:64, :])
                nc.vector.tensor_add(
                    dstT[64:128, h, sl], m2[64:128, :], m1[64:128, :])

            vt_init = set()

            def v_chain(b, sc, ssb):
                qt, kt, vt = get_qkv(b)
                if b not in vt_init:
                    vt_init.add(b)
                    nc.vector.memset(vt[:, :, :, HD:HD + 1], 1.0)
                xh = xstate[(b, sc)]
                kb = sc * (SCH // 128) + ssb
                psv = psA.tile([128, OSL], fp32, name="psv", tag="psA")
                for c in range(DCH):
                    nc.tensor.matmul(
                        psv[:],
                        xh[c // 8][:, c % 8, ssb * 128:(ssb + 1) * 128],
                        wv_sb[c // 8][:, c % 8, :],
                        start=(c == 0), stop=(c == DCH - 1))
                nc.vector.tensor_copy(
                    vt[:, kb, :, 0:HD],
                    psv[:].rearrange("p (h d) -> p h d", h=HPC))

            def proj_fillers(b):
                """Closure list emitting proj(b): fetches + chains."""
                items = []
                for sc in range(S // SCH):
                    if (b, sc) not in xstate:
                        items.append(lambda b=b, sc=sc: fetch_x(b, sc))
                    for h in range(HPC):
                        items.append(
                            lambda b=b, sc=sc, h=h: qk_chain(b, sc, wk_sb, 1, h))
                    for h in range(HPC):
                        items.append(
                            lambda b=b, sc=sc, h=h: qk_chain(b, sc, wq_sb, 0, h))
                    if sc + 1 < S // SCH:
                        items.append(lambda b=b, sc=sc: fetch_x(b, sc + 1))
                    elif b + 1 < B:
                        items.append(lambda b=b: fetch_x(b + 1, 0))
                    for ssb in range(SCH // 128):
                        items.append(
                            lambda b=b, sc=sc, ssb=ssb: v_chain(b, sc, ssb))
                return items

            # ---------- attention ----------
            expstate = {}
            atstate = {}

            def score_pair(b, u, j):
                """Two scores matmuls into one 2-bank PSUM tile + one exp."""
                qt, kt, vt = get_qkv(b)
                h, qc = u % HPC, u // HPC
                eh, jj = j // 4, j % 4
                if (u % 2, eh) not in expstate or expstate[(u % 2, eh)][0] != (b, u):
                    t = attp.tile([128, KB // 2, QC], bf16, name="expT",
                                  tag="expT")
                    expstate[(u % 2, eh)] = ((b, u), t)
                expT = expstate[(u % 2, eh)][1]
                pss = psB.tile([128, 2 * QC], fp32, name="pss", tag="psB")
                for i in range(2):
                    kb = eh * (KB // 2) + 2 * jj + i
                    nc.tensor.matmul(
                        pss[:, i * QC:(i + 1) * QC],
                        kt[:, h, kb * 128:(kb + 1) * 128],
                        qt[:, h, qc * QC:(qc + 1) * QC],
                        start=True, stop=True)
                nc.scalar.activation(
                    expT[:, 2 * jj:2 * jj + 2, :],
                    pss[:].rearrange("p (a n) -> p a n", a=2),
                    mybir.ActivationFunctionType.Exp,
                    scale=inv_sqrt_hd)

            def attnv_mms(b, u, s):
                """16 accumulating matmuls for q-subblock s of unit u."""
                qt, kt, vt = get_qkv(b)
                h = u % HPC
                e0 = expstate[(u % 2, 0)][1]
                e1 = expstate[(u % 2, 1)][1]
                if (b, u) not in atstate:
                    atstate[(b, u)] = atp.tile([128, QC], bf16, name="a_t",
                                               tag="a_t")
                a_t = atstate[(b, u)]
                pso = psCD.tile([128, HD + 1], fp32, name="pso", tag="psCD")
                for kb in range(KB):
                    eT = e0 if kb < KB // 2 else e1
                    nc.tensor.matmul(
                        pso[:],
                        eT[:, kb % (KB // 2), s * 128:(s + 1) * 128],
                        vt[:, kb, h, :],
                        start=(kb == 0), stop=(kb == KB - 1))
                return pso, a_t

            def finish(b, u, s, pso, a_t):
                """normalize + transpose + copy (+ bounce DMA / AG)."""
                h, qc = u % HPC, u // HPC
                rc = smalls.tile([128, 1], fp32, name="rc", tag="rc")
                nc.vector.reciprocal(rc[:], pso[:, HD:HD + 1])
                a_sb = smalls.tile([128, HD], bf16, name="a_sb", tag="a_sb")
                nc.vector.tensor_scalar_mul(a_sb[:], pso[:, 0:HD], rc[:])
                pst = psCD.tile([128, 128], bf16, name="pst", tag="psCD")
                nc.tensor.transpose(pst[:], a_sb[:], ident[:])
                nc.vector.tensor_copy(a_t[:, s * 128:(s + 1) * 128], pst[:])
                if s == 3:
                    ch, col0, last = _chunk_of(b, qc)
                    nc.sync.dma_start(
                        bounce[(b, ch)][h * HD:(h + 1) * HD, col0:col0 + QC],
                        a_t[:])
                    if h == HPC - 1 and last:
                        nc.gpsimd.collective_compute(
                            "AllGather",
                            mybir.AluOpType.bypass,
                            ins=[bounce[(b, ch)].opt()],
                            outs=[gath[(b, ch)].opt()],
                            replica_groups=[list(range(NCORES))],
                        )

            # ---------- wo stage ----------
            gtstate = {}

            def wo_prefetch(b, qq):
                """Load gathered [2048, 512] cols qq*512.. into 2 SBUF tiles."""
                ch, off, _ = _chunk_of(b, qq)
                gh = []
                for dh in range(2):
                    g = gtp.tile([128, DCH // 2, QC], bf16, name=f"gt{dh}",
                                 tag="gt")
                    nc.sync.dma_start(
                        g[:],
                        gath[(b, ch)][dh * 1024:(dh + 1) * 1024,
                                      off:off + QC]
                        .rearrange("(c p) n -> p c n", p=128))
                    gh.append(g)
                gtstate[(b, qq)] = gh

            def wo_chain(b, qq, oc):
                gh = gtstate[(b, qq)]
                psw = psA.tile([128, QC], fp32, name="psw", tag="psA")
                for c in range(DCH):
                    nc.tensor.matmul(
                        psw[:],
                        wo_sb[c // 8][:, c % 8, oc * 128:(oc + 1) * 128],
                        gh[c // 8][:, c % 8, :],
                        start=(c == 0), stop=(c == DCH - 1))
                out_t = otp.tile([128, QC], fp32, name="out_t", tag="out_t")
                nc.vector.tensor_copy(out_t[:], psw[:])
                col0 = b * S + qq * QC
                nc.scalar.dma_start(
                    outp[oc * 128:(oc + 1) * 128, col0:col0 + QC], out_t[:])

            def wo_fillers(b):
                items = [lambda b=b: wo_prefetch(b, 0),
                         lambda b=b: wo_prefetch(b, 1)]
                for qq in range(4):
                    for oc in range(OSL // 128):
                        items.append(
                            lambda b=b, qq=qq, oc=oc: wo_chain(b, qq, oc))
                    if qq + 2 < 4:
                        items.append(
                            lambda b=b, qq=qq: wo_prefetch(b, qq + 2))
                return items

            # ---------- the interleaved window ----------
            pending = [None]
            deferred_wo = []

            def flush_pending():
                if pending[0] is not None:
                    fin = pending[0]
                    pending[0] = None
                    fin()

            def window(b):
                fillers = []
                if b + 1 < B:
                    fillers.extend((0, f) for f in proj_fillers(b + 1))
                if b >= 1:
                    wof = wo_fillers(b - 1)
                    if b == B - 1:
                        deferred_wo.extend(wof[-2:])
                        wof = wof[:-2]
                    fillers.extend((0, f) for f in wof)
                nf = len(fillers)
                nsteps = 9 * 4
                emitted = [0]

                def pace(step):
                    target = (nf * (step + 1) + nsteps - 1) // nsteps
                    while (emitted[0] < min(target, nf)
                           and fillers[emitted[0]][0] <= step):
                        fillers[emitted[0]][1]()
                        emitted[0] += 1

                step = 0
                for u in range(9):
                    for s in range(4):
                        if u < 8:
                            score_pair(b, u, 2 * s)
                            score_pair(b, u, 2 * s + 1)
                        if u >= 1:
                            pso, a_t = attnv_mms(b, u - 1, s)
                            flush_pending()
                            pending[0] = (
                                lambda b=b, u=u - 1, s=s, pso=pso, a_t=a_t:
                                finish(b, u, s, pso, a_t))
                        elif u == 0 and s == 0:
                            # previous window's last finish, after 2 pairs
                            flush_pending()
                        pace(step)
                        step += 1

            # emission: proj(0) standalone, then the 4 windows, then wo(3)
            p0 = proj_fillers(0)
            for i, it in enumerate(p0):
                it()
                if i == 8:
                    wdma(wo_sb, woT, 0)
                elif i == 17:
                    wdma(wo_sb, woT, 1)
            for b in range(B):
                window(b)
            flush_pending()            # finish(7, 3) of batch 3 + its AG
            for f in deferred_wo:      # held-back wo(2) chains fill AG wait
                f()
            wo_prefetch(3, 0)
            wo_prefetch(3, 1)
            for oc in range(OSL // 128):
                wo_chain(3, 0, oc)
            wo_prefetch(3, 2)
            for oc in range(OSL // 128):
                wo_chain(3, 1, oc)
            wo_prefetch(3, 3)
            for qq in (2, 3):
                for oc in range(OSL // 128):
                    wo_chain(3, qq, oc)

    nc.compile()
    return nc


def _shard_inputs(x, freqs_cos, freqs_sin, wq, wk, wv, wo):
    xf = np.asarray(x, dtype=np.float32).reshape(ROWS, D)
    xT = np.ascontiguousarray(xf.T).astype(BF16)
    fcT = np.asarray(freqs_cos, dtype=np.float32).T  # [64, S]
    fsT = np.asarray(freqs_sin, dtype=np.float32).T
    cosd = np.ascontiguousarray(np.concatenate([fcT, fcT], 0))  # [128, S]
    sind = np.ascontiguousarray(np.concatenate([fsT, fsT], 0))
    # even indices (real half) then odd (imag half), per head
    perm = np.concatenate([np.arange(0, HD, 2), np.arange(1, HD, 2)])
    in_maps = []
    for c in range(NCORES):
        rows = slice(c * OSL, (c + 1) * OSL)
        wq_c = np.asarray(wq)[rows].reshape(HPC, HD, D)[:, perm, :].reshape(OSL, D)
        wk_c = np.asarray(wk)[rows].reshape(HPC, HD, D)[:, perm, :].reshape(OSL, D)
        in_maps.append({
            "xT": xT,
            "wqT": np.ascontiguousarray(wq_c.T).astype(BF16),
            "wkT": np.ascontiguousarray(wk_c.T).astype(BF16),
            "wvT": np.ascontiguousarray(np.asarray(wv)[rows].T).astype(BF16),
            "woT": np.ascontiguousarray(np.asarray(wo)[rows].T).astype(BF16),
            "cosd": cosd,
            "sind": sind,
        })
    return in_maps


def run(inputs, trace=False, trace_cores=None):
    """Build (cached), run on 8 cores; returns (full_output, BassKernelResults)."""
    global _NC_CACHE
    from concourse.bass_utils import run_bass_kernel_spmd
    if _NC_CACHE is None:
        _NC_CACHE = _build()
    in_maps = _shard_inputs(**inputs)
    res = run_bass_kernel_spmd(
        _NC_CACHE, in_maps, core_ids=list(range(NCORES)), trace=trace,
        trace_cores=trace_cores)
    parts = [np.ascontiguousarray(
        np.asarray(res.results[c]["out"], dtype=np.float32).T)
        for c in range(NCORES)]
    full = np.concatenate(parts, axis=1).reshape(B, S, D)
    return full, res


def kernel(x, freqs_cos, freqs_sin, wq, wk, wv, wo):
    full, _ = run(dict(x=x, freqs_cos=freqs_cos, freqs_sin=freqs_sin,
                       wq=wq, wk=wk, wv=wv, wo=wo))
    return full
